# revision 4
# baseline (speedup 1.0000x reference)
"""GCMCGraphConv Trainium2 kernel (8 NeuronCores, SPMD).

Sharding: destination-partitioned edge parallelism. Edges are sorted by
edge_dst on the host and split into 8 contiguous dst ranges (12500 dst
rows per core). The review-embedding table and feat table are replicated
to every core (bf16); each core gathers the rows for its own edges with
one batched indirect DMA per table per group (the SWDGE cost is ~1us
fixed per instruction + 0.34ns/row, so batching whole groups makes the
gathers nearly free), runs the per-edge review MLP on-chip, and
scatter-sums messages into its own dst rows with one-hot matmuls.
No collectives are needed; the host concatenates the 8 output shards.

The per-edge gating scalars pa=sigmoid(rfeat@prob_w), ra=sigmoid(
rfeat@score_w) are folded with cj*ci on the host into two per-edge
weights (wpa, wra) carried in emeta; this removes the sigmoid (and the
Gelu<->Sigmoid activation-table thrash) from the hot loop.

Per-core layout: a core's edges are grouped by blocks of 512 consecutive
destination rows ("groups"). Each group's edge list is padded to a
multiple of 128 (the SPMD program is identical across cores, so the
per-group capacity is the max count across the 8 cores). Padding edges
carry wpa=wra=0 and dst_rel=-1 so they contribute nothing.
"""

import os

import numpy as np

P = 128          # partitions / edge-tile size
FEAT = 256
REV_DIM = 128
GROUP = 512      # dst rows per scatter group
N_CORES = 8

_prog_cache = {}


def _build_program(rev_vocab, n_src, caps):
    from concourse import bass, tile, mybir, bacc

    NG = len(caps)
    n_slots = int(sum(caps))
    maxt = max(int(c) // P for c in caps)
    out_rows = NG * GROUP
    f32 = mybir.dt.float32
    bf16 = mybir.dt.bfloat16
    i32 = mybir.dt.int32

    nc = bacc.Bacc(None, target_bir_lowering=False, debug=False)

    rev_emb = nc.declare_dram_parameter("rev_emb", [rev_vocab, REV_DIM], bf16, isOutput=False)
    feat = nc.declare_dram_parameter("feat", [n_src, FEAT], bf16, isOutput=False)
    emeta = nc.declare_dram_parameter("emeta", [n_slots, 8], i32, isOutput=False)
    rw1t = nc.declare_dram_parameter("rw1t", [REV_DIM, FEAT], bf16, isOutput=False)
    rw2t = nc.declare_dram_parameter("rw2t", [P, 2 * FEAT], bf16, isOutput=False)
    rw3t = nc.declare_dram_parameter("rw3t", [P, 2 * FEAT], bf16, isOutput=False)
    lwt = nc.declare_dram_parameter("lwt", [P, 2 * FEAT], bf16, isOutput=False)
    linb = nc.declare_dram_parameter("linb", [1, FEAT], bf16, isOutput=False)
    ones1 = nc.declare_dram_parameter("ones1", [1, P], bf16, isOutput=False)
    ident = nc.declare_dram_parameter("ident", [P, P], bf16, isOutput=False)
    iota = nc.declare_dram_parameter("iota", [P, GROUP], f32, isOutput=False)
    out = nc.declare_dram_parameter("out", [out_rows, FEAT], f32, isOutput=True)

    AF = mybir.ActivationFunctionType
    OP = mybir.AluOpType

    with tile.TileContext(nc) as tc:
        with tc.tile_pool(name="const", bufs=1) as cpool, \
             tc.tile_pool(name="sb", bufs=4) as sb, \
             tc.tile_pool(name="stage", bufs=3) as stg, \
             tc.tile_pool(name="msgp", bufs=4) as msgp, \
             tc.tile_pool(name="ps", bufs=2, space="PSUM") as ps, \
             tc.tile_pool(name="psbc", bufs=2, space="PSUM") as psbc, \
             tc.tile_pool(name="psh", bufs=1, space="PSUM") as psh:

            c_rw1t = cpool.tile([REV_DIM, FEAT], bf16)
            nc.sync.dma_start(out=c_rw1t[:], in_=rw1t[:])
            c_rw2t = cpool.tile([P, 2 * FEAT], bf16)
            nc.sync.dma_start(out=c_rw2t[:], in_=rw2t[:])
            c_rw3t = cpool.tile([P, 2 * FEAT], bf16)
            nc.sync.dma_start(out=c_rw3t[:], in_=rw3t[:])
            c_lwt = cpool.tile([P, 2 * FEAT], bf16)
            nc.sync.dma_start(out=c_lwt[:], in_=lwt[:])
            c_linb = cpool.tile([1, FEAT], bf16)
            nc.sync.dma_start(out=c_linb[:], in_=linb[:])
            c_ones = cpool.tile([1, P], bf16)
            nc.sync.dma_start(out=c_ones[:], in_=ones1[:])
            c_id = cpool.tile([P, P], bf16)
            nc.sync.dma_start(out=c_id[:], in_=ident[:])
            c_iota = cpool.tile([P, GROUP], f32)
            nc.sync.dma_start(out=c_iota[:], in_=iota[:])

            slot = 0

            def pass1(g, slot):
                ntile = int(caps[g]) // P
                em_g = stg.tile([P, maxt, 8], i32, tag="em")
                nc.sync.dma_start(
                    out=em_g[:, 0:ntile, :],
                    in_=emeta[slot:slot + ntile * P, :].rearrange(
                        "(n p) d -> p n d", p=P))
                ems = [em_g[:, t, :] for t in range(ntile)]
                # indirect DMA only supports one gathered row per partition
                # per instruction, so gather tile-by-tile
                rfe_g = stg.tile([P, maxt, REV_DIM], bf16, tag="rfe")
                fte_g = stg.tile([P, maxt, FEAT], bf16, tag="fte")
                for t in range(ntile):
                    nc.gpsimd.indirect_dma_start(
                        out=rfe_g[:, t, :], out_offset=None, in_=rev_emb[:],
                        in_offset=bass.IndirectOffsetOnAxis(
                            ap=em_g[:, t, 0:1], axis=0))
                    nc.gpsimd.indirect_dma_start(
                        out=fte_g[:, t, :], out_offset=None, in_=feat[:],
                        in_offset=bass.IndirectOffsetOnAxis(
                            ap=em_g[:, t, 1:2], axis=0))
                rfbs = []
                chunk_of, off_of = {}, {}
                t = 0
                pi = 0
                while t < ntile:
                    ct = 2 if t + 1 < ntile else 1
                    W = ct * P
                    p_rT = ps.tile([P, 2 * P], bf16, tag="pA")
                    for k in range(ct):
                        nc.tensor.transpose(out=p_rT[:, k * P:(k + 1) * P],
                                            in_=rfe_g[:, t + k, :], identity=c_id[:])
                    rT = sb.tile([P, 2 * P], bf16, tag="rT")
                    nc.vector.tensor_copy(out=rT[:, 0:W], in_=p_rT[:, 0:W])
                    a1 = []
                    for m in range(2):
                        pa1 = psbc.tile([P, 2 * P], f32, tag="pBC")
                        nc.tensor.matmul(out=pa1[:, 0:W],
                                         lhsT=c_rw1t[:, m * P:(m + 1) * P],
                                         rhs=rT[:, 0:W], start=True, stop=True)
                        a1m = sb.tile([P, 2 * P], bf16, tag=f"a1_{m}")
                        nc.scalar.activation(out=a1m[:, 0:W], in_=pa1[:, 0:W], func=AF.Gelu)
                        a1.append(a1m)
                    a2 = []
                    for m in range(2):
                        pa2 = psbc.tile([P, 2 * P], f32, tag="pBC")
                        for j in range(2):
                            nc.tensor.matmul(
                                out=pa2[:, 0:W],
                                lhsT=c_rw2t[:, j * FEAT + m * P: j * FEAT + (m + 1) * P],
                                rhs=a1[j][:, 0:W], start=(j == 0), stop=(j == 1))
                        a2m = sb.tile([P, 2 * P], bf16, tag=f"a2_{m}")
                        nc.scalar.activation(out=a2m[:, 0:W], in_=pa2[:, 0:W], func=AF.Gelu)
                        a2.append(a2m)
                    p_rf = ps.tile([P, 2 * FEAT], f32, tag="pDE")
                    for k in range(ct):
                        for j in range(2):
                            nc.tensor.matmul(
                                out=p_rf[:, k * FEAT:(k + 1) * FEAT],
                                lhsT=a2[j][:, k * P:(k + 1) * P],
                                rhs=c_rw3t[:, j * FEAT:(j + 1) * FEAT],
                                start=(j == 0), stop=(j == 1))
                    rfb = stg.tile([P, 2 * FEAT], bf16, tag=f"rfb{pi}")
                    nc.scalar.activation(out=rfb[:, 0:ct * FEAT], in_=p_rf[:, 0:ct * FEAT],
                                         func=AF.Copy)
                    rfbs.append(rfb)
                    for k in range(ct):
                        chunk_of[t + k] = pi
                        off_of[t + k] = k
                    t += ct
                    pi += 1
                return dict(g=g, ntile=ntile, ems=ems, fte_g=fte_g, rfbs=rfbs,
                            chunk_of=chunk_of, off_of=off_of)

            def pass2(st):
                g = st["g"]
                ntile = st["ntile"]
                ems, fte_g, rfbs = st["ems"], st["fte_g"], st["rfbs"]
                chunk_of, off_of = st["chunk_of"], st["off_of"]
                h0 = psh.tile([P, GROUP], f32, tag="h0")
                h1 = psh.tile([P, GROUP], f32, tag="h1")
                for t in range(ntile):
                    em = ems[t]
                    t2 = msgp.tile([P, FEAT], bf16, tag="t2")
                    nc.vector.tensor_scalar(
                        out=t2[:],
                        in0=rfbs[chunk_of[t]][:, off_of[t] * FEAT:(off_of[t] + 1) * FEAT],
                        scalar1=em[:, 3:4].bitcast(f32), scalar2=None, op0=OP.mult)
                    mm = msgp.tile([P, FEAT], bf16, tag="mm")
                    nc.vector.scalar_tensor_tensor(
                        out=mm[:], in0=fte_g[:, t, :],
                        scalar=em[:, 2:3].bitcast(f32), in1=t2[:],
                        op0=OP.mult, op1=OP.add)
                    S = msgp.tile([P, GROUP], bf16, tag="S")
                    nc.vector.tensor_scalar(out=S[:], in0=c_iota[:],
                                            scalar1=em[:, 4:5].bitcast(f32),
                                            scalar2=None, op0=OP.is_equal)
                    nc.tensor.matmul(out=h0[:], lhsT=mm[:, 0:P], rhs=S[:],
                                     start=(t == 0), stop=(t == ntile - 1))
                    nc.tensor.matmul(out=h1[:], lhsT=mm[:, P:FEAT], rhs=S[:],
                                     start=(t == 0), stop=(t == ntile - 1))
                hs = sb.tile([P, 2 * GROUP], bf16, tag="hs")
                nc.vector.tensor_copy(out=hs[:, 0:GROUP], in_=h0[:])
                nc.vector.tensor_copy(out=hs[:, GROUP:2 * GROUP], in_=h1[:])
                for b in range(GROUP // P):
                    po = ps.tile([P, 2 * FEAT], f32, tag="pDE")
                    nc.tensor.matmul(out=po[:, 0:FEAT], lhsT=hs[:, b * P:(b + 1) * P],
                                     rhs=c_lwt[:, 0:FEAT], start=True, stop=False)
                    nc.tensor.matmul(out=po[:, 0:FEAT], lhsT=hs[:, GROUP + b * P:GROUP + (b + 1) * P],
                                     rhs=c_lwt[:, FEAT:2 * FEAT], start=False, stop=False)
                    nc.tensor.matmul(out=po[:, 0:FEAT], lhsT=c_ones[0:1, :], rhs=c_linb[0:1, :],
                                     start=False, stop=True)
                    ot = sb.tile([P, FEAT], f32, tag="ot")
                    nc.vector.tensor_copy(out=ot[:], in_=po[:, 0:FEAT])
                    nc.sync.dma_start(
                        out=out[g * GROUP + b * P: g * GROUP + (b + 1) * P, :],
                        in_=ot[:])

            prev = None
            for g in range(NG):
                st = pass1(g, slot)
                slot += int(caps[g])
                if prev is not None:
                    pass2(prev)
                prev = st
            pass2(prev)
    nc.compile()
    return nc


def kernel(**inputs):
    import ml_dtypes
    from concourse.bass_utils import run_bass_kernel_spmd

    feat = np.asarray(inputs["feat"], dtype=np.float32)
    cj = np.asarray(inputs["cj"], dtype=np.float32)
    ci = np.asarray(inputs["ci"], dtype=np.float32)
    edge_src = np.asarray(inputs["edge_src"]).astype(np.int64)
    edge_dst = np.asarray(inputs["edge_dst"]).astype(np.int64)
    review_id = np.asarray(inputs["review_id"]).astype(np.int64)
    rev_emb = np.asarray(inputs["review_emb"], dtype=np.float32)
    prob_w = np.asarray(inputs["prob_w"], dtype=np.float32)
    score_w = np.asarray(inputs["score_w"], dtype=np.float32)
    rw1 = np.asarray(inputs["rw1"], dtype=np.float32)
    rw2 = np.asarray(inputs["rw2"], dtype=np.float32)
    rw3 = np.asarray(inputs["rw3"], dtype=np.float32)
    lin_w = np.asarray(inputs["lin_w"], dtype=np.float32)
    lin_b = np.asarray(inputs["lin_b"], dtype=np.float32)

    n_src = feat.shape[0]
    n_dst = ci.shape[0]
    rev_vocab = rev_emb.shape[0]
    dst_per_core = n_dst // N_CORES
    ng = -(-dst_per_core // GROUP)
    bf = ml_dtypes.bfloat16

    order = np.argsort(edge_dst, kind="stable")
    s_src = edge_src[order]
    s_dst = edge_dst[order]
    s_rev = review_id[order]
    s_w = (cj[s_src, 0] * ci[s_dst, 0]).astype(np.float32)
    # host-side gating: pa/ra sigmoids folded into the per-edge weights
    rfeat = rev_emb[s_rev]
    pa = 1.0 / (1.0 + np.exp(-(rfeat @ prob_w[0])))
    ra = 1.0 / (1.0 + np.exp(-(rfeat @ score_w[0])))
    s_wpa = (pa * s_w).astype(np.float32)
    s_wra = (ra * s_w).astype(np.float32)

    core_of = s_dst // dst_per_core
    core_start = np.searchsorted(core_of, np.arange(N_CORES), side="left")
    core_end = np.searchsorted(core_of, np.arange(N_CORES), side="right")

    counts = np.zeros((N_CORES, ng), dtype=np.int64)
    group_starts = np.zeros((N_CORES, ng), dtype=np.int64)
    for c in range(N_CORES):
        lo, hi = core_start[c], core_end[c]
        dloc = s_dst[lo:hi] - c * dst_per_core
        gid = dloc // GROUP
        counts[c] = np.bincount(gid, minlength=ng)
        group_starts[c] = lo + np.concatenate(([0], np.cumsum(counts[c])[:-1]))

    caps = (np.maximum(1, -(-counts.max(axis=0) // P)) * P).astype(np.int64)
    n_slots = int(caps.sum())

    consts = dict(
        rw1t=np.ascontiguousarray(rw1.T).astype(bf),
        rw2t=np.ascontiguousarray(
            np.concatenate([rw2.T[0:P, :], rw2.T[P:2 * P, :]], axis=1)).astype(bf),
        rw3t=np.ascontiguousarray(
            np.concatenate([rw3.T[0:P, :], rw3.T[P:2 * P, :]], axis=1)).astype(bf),
        lwt=np.ascontiguousarray(
            np.concatenate([lin_w.T[0:P, :], lin_w.T[P:2 * P, :]], axis=1)).astype(bf),
        linb=lin_b.reshape(1, FEAT).astype(bf),
        ones1=np.ones((1, P), dtype=bf),
        ident=np.eye(P, dtype=bf),
        iota=np.broadcast_to(np.arange(GROUP, dtype=np.float32), (P, GROUP)).copy(),
    )
    rev16 = rev_emb.astype(bf)
    feat16 = feat.astype(bf)

    in_maps = []
    for c in range(N_CORES):
        emeta = np.zeros((n_slots, 8), dtype=np.int32)
        emeta[:, 4] = np.float32(-1.0).view(np.int32)
        slot = 0
        for g in range(ng):
            n = int(counts[c, g])
            lo = int(group_starts[c, g])
            emeta[slot:slot + n, 0] = s_rev[lo:lo + n]
            emeta[slot:slot + n, 1] = s_src[lo:lo + n]
            emeta[slot:slot + n, 2] = s_wpa[lo:lo + n].view(np.int32)
            emeta[slot:slot + n, 3] = s_wra[lo:lo + n].view(np.int32)
            emeta[slot:slot + n, 4] = (
                (s_dst[lo:lo + n] - c * dst_per_core - g * GROUP)
                .astype(np.float32).view(np.int32))
            slot += int(caps[g])
        im = dict(rev_emb=rev16, feat=feat16, emeta=emeta, **consts)
        in_maps.append(im)

    key = (rev_vocab, n_src, tuple(int(x) for x in caps))
    if key not in _prog_cache:
        _prog_cache[key] = _build_program(rev_vocab, n_src, caps)
    nc = _prog_cache[key]

    trace = bool(os.environ.get("BASS_KERNEL_TRACE"))
    res = run_bass_kernel_spmd(nc, in_maps, core_ids=list(range(N_CORES)),
                               trace=trace)
    global last_results
    last_results = res
    out = np.concatenate(
        [res.results[c]["out"][:dst_per_core] for c in range(N_CORES)], axis=0)
    return out.astype(np.float32)


last_results = None


# revision 8
# speedup vs baseline: 3.0993x; 3.0993x over previous
"""GCMCGraphConv Trainium2 kernel (8 NeuronCores, SPMD).

Design notes (v3):

Sharding: destination-partitioned edge parallelism. Edges are sorted by
edge_dst on the host; consecutive nonzero-degree dst rows are greedily
packed into groups of <=CAP edges and <=128 rows, and groups are dealt
round-robin to the 8 cores. Every group has a fixed capacity of CAP
slots (3 tiles of 128), so the SPMD program is identical across cores;
padding slots carry zero weights.

The per-edge table rows are pre-gathered ON THE HOST into dense per-slot
streams (the SWDGE indirect-DMA path costs ~1us of gpsimd time per 128
gathered rows on this target, which would dominate the kernel; dense
streams move the same bytes at full DMA bandwidth):
  - rth  [128, n_slots] bf16: review embedding rows, pre-TRANSPOSED so
    the MLP's first matmul can consume them directly (no on-chip
    transposes).
  - fth  [n_slots, 256] bf16: feat rows per slot.
The gating scalars pa=sigmoid(rfeat@prob_w), ra=sigmoid(rfeat@score_w)
are folded with cj*ci on the host into per-slot weights wpa/wra.

On-chip per tile (128 slots): MLP layer1 (bf16) + Gelu -> fp8e5, layer2
as ONE DoubleRow fp8 matmul (256-deep contraction in half the cycles) +
Gelu -> bf16. Layer3 (rw3) is NOT applied per edge: messages are
scatter-summed first and rw3 is folded into the final linear
(out_rf = (lin_w@rw3) @ G with G = sum_e wra_e * a2_e one-hot scattered,
which is exact because everything after the second Gelu is linear).

Scatter: per tile two scaled one-hot matrices S_a = onehot(dst)*wpa,
S_b = onehot(dst)*wra (one DVE op each) feed four 128-col matmuls
accumulating hTa = sum wpa*feat (x) onehot and G = sum wra*a2 (x) onehot
in PSUM over the group's 3 tiles. Per group, four 256-col matmuls apply
lin_w / lin_w@rw3 to produce out[dst0:dst0+128, 256]; the bias and the
zero-degree rows are applied on the host during reassembly.
"""

import os

import numpy as np

P = 128
FEAT = 256
REV_DIM = 128
CAP = 384        # slots per group (3 tiles), <=128 dst rows per group
TPG = CAP // P   # tiles per group = 3
GB = 8           # groups per gather/stage batch
N_CORES = 8

_prog_cache = {}


def _build_program(ng):
    from concourse import tile, mybir, bacc

    n_slots = ng * CAP
    T = ng * TPG                 # tiles per core
    nb = ng // GB                # stage batches
    SLOTS_B = CAP * GB           # slots per batch
    TB = TPG * GB                # tiles per batch
    PAIRS_B = TB // 2
    f32 = mybir.dt.float32
    bf16 = mybir.dt.bfloat16
    f16 = mybir.dt.float16
    fp8 = mybir.dt.float8e5
    MM = mybir.MatmulPerfMode

    nc = bacc.Bacc(None, target_bir_lowering=False, debug=False)

    rth = nc.declare_dram_parameter("rth", [P, n_slots], bf16, isOutput=False)
    fth = nc.declare_dram_parameter("fth", [n_slots, FEAT], bf16, isOutput=False)
    wpa = nc.declare_dram_parameter("wpa", [P, T], f32, isOutput=False)
    wra = nc.declare_dram_parameter("wra", [P, T], f32, isOutput=False)
    dstr = nc.declare_dram_parameter("dstr", [P, T], f32, isOutput=False)
    rw1t = nc.declare_dram_parameter("rw1t", [REV_DIM, FEAT], bf16, isOutput=False)
    rw2t8 = nc.declare_dram_parameter("rw2t8", [P, 2, FEAT], fp8, isOutput=False)
    w3lt = nc.declare_dram_parameter("w3lt", [P, 2, FEAT], bf16, isOutput=False)
    lwt = nc.declare_dram_parameter("lwt", [P, 2, FEAT], bf16, isOutput=False)
    iota = nc.declare_dram_parameter("iota", [P, P], f16, isOutput=False)
    outd = nc.declare_dram_parameter("outd", [ng * P, FEAT], f32, isOutput=True)

    AF = mybir.ActivationFunctionType
    OP = mybir.AluOpType

    with tile.TileContext(nc) as tc:
        with tc.tile_pool(name="const", bufs=1) as cpool, \
             tc.tile_pool(name="stage", bufs=2) as stg, \
             tc.tile_pool(name="a1p", bufs=3) as a1pool, \
             tc.tile_pool(name="msg", bufs=6) as msg, \
             tc.tile_pool(name="drain", bufs=3) as drn, \
             tc.tile_pool(name="ot", bufs=2) as otp, \
             tc.tile_pool(name="psA", bufs=2, space="PSUM") as psA, \
             tc.tile_pool(name="psB", bufs=2, space="PSUM") as psB, \
             tc.tile_pool(name="psC", bufs=2, space="PSUM") as psC, \
             tc.tile_pool(name="psO", bufs=2, space="PSUM") as psO:

            c_rw1t = cpool.tile([REV_DIM, FEAT], bf16)
            nc.sync.dma_start(out=c_rw1t[:], in_=rw1t[:])
            c_rw2t8 = cpool.tile([P, 2, FEAT], fp8)
            nc.sync.dma_start(out=c_rw2t8[:], in_=rw2t8[:])
            c_w3lt = cpool.tile([P, 2, FEAT], bf16)
            nc.sync.dma_start(out=c_w3lt[:], in_=w3lt[:])
            c_lwt = cpool.tile([P, 2, FEAT], bf16)
            nc.sync.dma_start(out=c_lwt[:], in_=lwt[:])
            c_iota = cpool.tile([P, P], f16)
            nc.sync.dma_start(out=c_iota[:], in_=iota[:])
            c_wpa = cpool.tile([P, T], f32)
            nc.sync.dma_start(out=c_wpa[:], in_=wpa[:])
            c_wra = cpool.tile([P, T], f32)
            nc.sync.dma_start(out=c_wra[:], in_=wra[:])
            c_dstr = cpool.tile([P, T], f32)
            nc.sync.dma_start(out=c_dstr[:], in_=dstr[:])

            for b in range(nb):
                s0 = b * SLOTS_B
                rts = stg.tile([P, SLOTS_B], bf16, tag="rts")
                nc.sync.dma_start(out=rts[:], in_=rth[:, s0:s0 + SLOTS_B])
                fts = stg.tile([P, TB, FEAT], bf16, tag="fts")
                nc.sync.dma_start(
                    out=fts[:],
                    in_=fth[s0:s0 + SLOTS_B, :].rearrange("(n p) d -> p n d", p=P))
                a2b = stg.tile([P, TB, FEAT], bf16, tag="a2b")

                for pi in range(PAIRS_B):
                    a1ps = psA.tile([P, 2, FEAT], f32, tag="a1ps")
                    for m in range(2):
                        nc.tensor.matmul(
                            out=a1ps[:, m, :],
                            lhsT=c_rw1t[:, m * P:(m + 1) * P],
                            rhs=rts[:, pi * 256:(pi + 1) * 256],
                            start=True, stop=True)
                    a1sb = a1pool.tile([P, 2, FEAT], fp8, tag="a1sb")
                    nc.scalar.activation(out=a1sb[:], in_=a1ps[:], func=AF.Gelu)
                    a2ps = psB.tile([P, 2, FEAT], f32, tag="a2ps")
                    for k in range(2):
                        nc.tensor.matmul(
                            out=a2ps[:, k, :],
                            lhsT=a1sb[:, :, k * P:(k + 1) * P],
                            rhs=c_rw2t8[:],
                            start=True, stop=True, perf_mode=MM.DoubleRow)
                    nc.scalar.activation(out=a2b[:, pi * 2:(pi + 1) * 2, :],
                                         in_=a2ps[:], func=AF.Gelu)

                po = None
                for gi in range(GB):
                    g = b * GB + gi
                    acc = psC.tile([P, 2, FEAT], f32, tag="acc")
                    # PSUM accumulation chains must be contiguous per bank
                    # region on HW: build all S tiles first, then run the
                    # four region chains back to back.
                    sas, sbs = [], []
                    for gt in range(TPG):
                        tg = b * TB + gi * TPG + gt
                        sa = msg.tile([P, P], bf16, tag="sa")
                        nc.vector.tensor_scalar(
                            out=sa[:], in0=c_iota[:],
                            scalar1=c_dstr[:, tg:tg + 1],
                            scalar2=c_wpa[:, tg:tg + 1],
                            op0=OP.is_equal, op1=OP.mult)
                        sas.append(sa)
                        sb_ = msg.tile([P, P], bf16, tag="sb")
                        nc.vector.tensor_scalar(
                            out=sb_[:], in0=c_iota[:],
                            scalar1=c_dstr[:, tg:tg + 1],
                            scalar2=c_wra[:, tg:tg + 1],
                            op0=OP.is_equal, op1=OP.mult)
                        sbs.append(sb_)
                    for f in range(2):
                        for gt in range(TPG):
                            ti = gi * TPG + gt
                            nc.tensor.matmul(
                                out=acc[:, 0, f * P:(f + 1) * P],
                                lhsT=fts[:, ti, f * P:(f + 1) * P],
                                rhs=sas[gt][:],
                                start=(gt == 0), stop=(gt == TPG - 1))
                    for j in range(2):
                        for gt in range(TPG):
                            ti = gi * TPG + gt
                            nc.tensor.matmul(
                                out=acc[:, 1, j * P:(j + 1) * P],
                                lhsT=a2b[:, ti, j * P:(j + 1) * P],
                                rhs=sbs[gt][:],
                                start=(gt == 0), stop=(gt == TPG - 1))
                    hta = drn.tile([P, FEAT], bf16, tag="hta")
                    nc.vector.tensor_copy(out=hta[:], in_=acc[:, 0, :])
                    gsb = drn.tile([P, FEAT], bf16, tag="gsb")
                    nc.vector.tensor_copy(out=gsb[:], in_=acc[:, 1, :])
                    if gi % 2 == 0:
                        po = psO.tile([P, 2, FEAT], f32, tag="po")
                    for f in range(2):
                        nc.tensor.matmul(
                            out=po[:, gi % 2, :],
                            lhsT=hta[:, f * P:(f + 1) * P],
                            rhs=c_lwt[:, f, :],
                            start=(f == 0), stop=False)
                    for j in range(2):
                        nc.tensor.matmul(
                            out=po[:, gi % 2, :],
                            lhsT=gsb[:, j * P:(j + 1) * P],
                            rhs=c_w3lt[:, j, :],
                            start=False, stop=(j == 1))
                    if gi % 2 == 1:
                        ot = otp.tile([P, 2, FEAT], f32, tag="ot")
                        nc.scalar.activation(out=ot[:], in_=po[:], func=AF.Copy)
                        g0 = g - 1
                        nc.sync.dma_start(
                            out=outd[g0 * P:(g0 + 2) * P, :].rearrange(
                                "(n p) d -> p n d", p=P),
                            in_=ot[:])
    nc.compile()
    return nc


def kernel(**inputs):
    import ml_dtypes
    from concourse.bass_utils import run_bass_kernel_spmd

    feat = np.asarray(inputs["feat"], dtype=np.float32)
    cj = np.asarray(inputs["cj"], dtype=np.float32)
    ci = np.asarray(inputs["ci"], dtype=np.float32)
    edge_src = np.asarray(inputs["edge_src"]).astype(np.int64)
    edge_dst = np.asarray(inputs["edge_dst"]).astype(np.int64)
    review_id = np.asarray(inputs["review_id"]).astype(np.int64)
    rev_emb = np.asarray(inputs["review_emb"], dtype=np.float32)
    prob_w = np.asarray(inputs["prob_w"], dtype=np.float32)
    score_w = np.asarray(inputs["score_w"], dtype=np.float32)
    rw1 = np.asarray(inputs["rw1"], dtype=np.float32)
    rw2 = np.asarray(inputs["rw2"], dtype=np.float32)
    rw3 = np.asarray(inputs["rw3"], dtype=np.float32)
    lin_w = np.asarray(inputs["lin_w"], dtype=np.float32)
    lin_b = np.asarray(inputs["lin_b"], dtype=np.float32)

    n_dst = ci.shape[0]
    bf = ml_dtypes.bfloat16

    order = np.argsort(edge_dst, kind="stable")
    s_src = edge_src[order]
    s_dst = edge_dst[order]
    s_rev = review_id[order]
    s_w = (cj[s_src, 0] * ci[s_dst, 0]).astype(np.float32)
    rfeat = rev_emb[s_rev]
    pa = 1.0 / (1.0 + np.exp(-(rfeat @ prob_w[0])))
    ra = 1.0 / (1.0 + np.exp(-(rfeat @ score_w[0])))
    s_wpa = (pa * s_w).astype(np.float32)
    s_wra = (ra * s_w).astype(np.float32)

    # nonzero dst rows in sorted order, with degree and edge offsets
    uniq_dst, deg = np.unique(s_dst, return_counts=True)
    row_end = np.cumsum(deg)            # edge offset after each row
    row_start = row_end - deg
    nrows_tot = len(uniq_dst)

    # greedy pack consecutive rows into groups: <=CAP edges, <=128 rows
    grp_row0 = []      # first row index (into uniq_dst) of each group
    grp_nrows = []
    i = 0
    while i < nrows_tot:
        base = row_start[i]
        # farthest j with row_end[j] - base <= CAP
        j = np.searchsorted(row_end, base + CAP, side="right") - 1
        j = min(max(j, i), i + P - 1)
        grp_row0.append(i)
        grp_nrows.append(j - i + 1)
        i = j + 1
    ngroups = len(grp_row0)
    ng = -(-ngroups // N_CORES)
    ng = -(-ng // GB) * GB             # groups per core, multiple of GB
    n_slots = ng * CAP
    T = ng * TPG

    # deal groups round-robin to cores: group g -> core g % 8, index g // 8
    rev16 = rev_emb.astype(bf)
    feat16 = feat.astype(bf)

    consts = dict(
        rw1t=np.ascontiguousarray(rw1.T).astype(bf),
        rw2t8=np.ascontiguousarray(
            rw2.T.reshape(2, P, FEAT).transpose(1, 0, 2)).astype(ml_dtypes.float8_e5m2),
        w3lt=np.ascontiguousarray(
            (lin_w @ rw3).T.reshape(2, P, FEAT).transpose(1, 0, 2)).astype(bf),
        lwt=np.ascontiguousarray(
            lin_w.T.reshape(2, P, FEAT).transpose(1, 0, 2)).astype(bf),
        iota=np.broadcast_to(np.arange(P, dtype=np.float16), (P, P)).copy(),
    )

    in_maps = []
    core_meta = []
    for c in range(N_CORES):
        gl = list(range(c, ngroups, N_CORES))[:ng]
        slot_rev = np.zeros(n_slots, dtype=np.int64)
        slot_src = np.zeros(n_slots, dtype=np.int64)
        wpa_s = np.zeros(n_slots, dtype=np.float32)
        wra_s = np.zeros(n_slots, dtype=np.float32)
        dst_s = np.full(n_slots, -1.0, dtype=np.float32)
        rows_all = []
        pos_all = []
        for k, g in enumerate(gl):
            r0, nr = grp_row0[g], grp_nrows[g]
            e0, e1 = row_start[r0], row_end[r0 + nr - 1]
            n = e1 - e0
            s0 = k * CAP
            slot_rev[s0:s0 + n] = s_rev[e0:e1]
            slot_src[s0:s0 + n] = s_src[e0:e1]
            wpa_s[s0:s0 + n] = s_wpa[e0:e1]
            wra_s[s0:s0 + n] = s_wra[e0:e1]
            dst_s[s0:s0 + n] = (
                np.searchsorted(uniq_dst[r0:r0 + nr], s_dst[e0:e1])
            ).astype(np.float32)
            rows_all.append(uniq_dst[r0:r0 + nr])
            pos_all.append(k * P + np.arange(nr))
        rth = np.ascontiguousarray(rev16[slot_rev].T)           # [128, n_slots]
        fth = np.ascontiguousarray(feat16[slot_src])            # [n_slots, 256]
        im = dict(
            rth=rth, fth=fth,
            wpa=np.ascontiguousarray(wpa_s.reshape(T, P).T),
            wra=np.ascontiguousarray(wra_s.reshape(T, P).T),
            dstr=np.ascontiguousarray(dst_s.reshape(T, P).T),
            **consts)
        in_maps.append(im)
        core_meta.append((np.concatenate(rows_all) if rows_all else np.zeros(0, dtype=np.int64),
                          np.concatenate(pos_all) if pos_all else np.zeros(0, dtype=np.int64)))

    global last_inmaps, last_meta
    last_inmaps = in_maps
    last_meta = dict(ng=ng, T=T, n_slots=n_slots, core_meta=core_meta,
                     uniq_dst=uniq_dst, grp_row0=grp_row0, grp_nrows=grp_nrows)

    if ng not in _prog_cache:
        _prog_cache[ng] = _build_program(ng)
    nc = _prog_cache[ng]

    trace = bool(os.environ.get("BASS_KERNEL_TRACE"))
    res = run_bass_kernel_spmd(nc, in_maps, core_ids=list(range(N_CORES)),
                               trace=trace)
    global last_results
    last_results = res

    out = np.broadcast_to(lin_b, (n_dst, FEAT)).astype(np.float32).copy()
    for c in range(N_CORES):
        rows, pos = core_meta[c]
        if len(rows):
            out[rows] = res.results[c]["outd"][pos] + lin_b
    return out


last_results = None
last_inmaps = None
last_meta = None


# revision 12
# speedup vs baseline: 3.1090x; 1.0031x over previous
"""GCMCGraphConv Trainium2 kernel (8 NeuronCores, SPMD).

Design notes (v3):

Sharding: destination-partitioned edge parallelism. Edges are sorted by
edge_dst on the host; consecutive nonzero-degree dst rows are greedily
packed into groups of <=CAP edges and <=128 rows, and groups are dealt
round-robin to the 8 cores. Every group has a fixed capacity of CAP
slots (3 tiles of 128), so the SPMD program is identical across cores;
padding slots carry zero weights.

The per-edge table rows are pre-gathered ON THE HOST into dense per-slot
streams (the SWDGE indirect-DMA path costs ~1us of gpsimd time per 128
gathered rows on this target, which would dominate the kernel; dense
streams move the same bytes at full DMA bandwidth):
  - rth  [128, n_slots] bf16: review embedding rows, pre-TRANSPOSED so
    the MLP's first matmul can consume them directly (no on-chip
    transposes).
  - fth  [n_slots, 256] bf16: feat rows per slot.
The gating scalars pa=sigmoid(rfeat@prob_w), ra=sigmoid(rfeat@score_w)
are folded with cj*ci on the host into per-slot weights wpa/wra.

On-chip per tile (128 slots): MLP layer1 (bf16) + Gelu -> fp8e5, layer2
as ONE DoubleRow fp8 matmul (256-deep contraction in half the cycles) +
Gelu -> bf16. Layer3 (rw3) is NOT applied per edge: messages are
scatter-summed first and rw3 is folded into the final linear
(out_rf = (lin_w@rw3) @ G with G = sum_e wra_e * a2_e one-hot scattered,
which is exact because everything after the second Gelu is linear).

Scatter: per tile two scaled one-hot matrices S_a = onehot(dst)*wpa,
S_b = onehot(dst)*wra (one DVE op each) feed four 128-col matmuls
accumulating hTa = sum wpa*feat (x) onehot and G = sum wra*a2 (x) onehot
in PSUM over the group's 3 tiles. Per group, four 256-col matmuls apply
lin_w / lin_w@rw3 to produce out[dst0:dst0+128, 256]; the bias and the
zero-degree rows are applied on the host during reassembly.
"""

import os

import numpy as np

P = 128
FEAT = 256
REV_DIM = 128
CAP = 384        # slots per group (3 tiles), <=128 dst rows per group
TPG = CAP // P   # tiles per group = 3
GB = 8           # groups per gather/stage batch
N_CORES = 8

_prog_cache = {}


def _build_program(ng):
    from concourse import tile, mybir, bacc

    n_slots = ng * CAP
    T = ng * TPG                 # tiles per core
    nb = ng // GB                # stage batches
    SLOTS_B = CAP * GB           # slots per batch
    TB = TPG * GB                # tiles per batch
    PAIRS_B = TB // 2
    f32 = mybir.dt.float32
    bf16 = mybir.dt.bfloat16
    f16 = mybir.dt.float16
    fp8 = mybir.dt.float8e5
    MM = mybir.MatmulPerfMode

    nc = bacc.Bacc(None, target_bir_lowering=False, debug=False)

    rth = nc.declare_dram_parameter("rth", [P, n_slots], bf16, isOutput=False)
    fth = nc.declare_dram_parameter("fth", [n_slots, FEAT], bf16, isOutput=False)
    wpa = nc.declare_dram_parameter("wpa", [P, T], f32, isOutput=False)
    wra = nc.declare_dram_parameter("wra", [P, T], f32, isOutput=False)
    dstr = nc.declare_dram_parameter("dstr", [P, T], f32, isOutput=False)
    rw1t = nc.declare_dram_parameter("rw1t", [REV_DIM, FEAT], bf16, isOutput=False)
    rw2t8 = nc.declare_dram_parameter("rw2t8", [P, 2, FEAT], fp8, isOutput=False)
    w3lt = nc.declare_dram_parameter("w3lt", [P, 2, FEAT], bf16, isOutput=False)
    lwt = nc.declare_dram_parameter("lwt", [P, 2, FEAT], bf16, isOutput=False)
    iota = nc.declare_dram_parameter("iota", [P, P], bf16, isOutput=False)
    outd = nc.declare_dram_parameter("outd", [ng * P, FEAT], bf16, isOutput=True)

    AF = mybir.ActivationFunctionType
    OP = mybir.AluOpType

    with tile.TileContext(nc) as tc:
        with tc.tile_pool(name="const", bufs=1) as cpool, \
             tc.tile_pool(name="stage", bufs=2) as stg, \
             tc.tile_pool(name="a1p", bufs=3) as a1pool, \
             tc.tile_pool(name="msg", bufs=6) as msg, \
             tc.tile_pool(name="drain", bufs=3) as drn, \
             tc.tile_pool(name="ot", bufs=2) as otp, \
             tc.tile_pool(name="psA", bufs=3, space="PSUM") as psA, \
             tc.tile_pool(name="psB", bufs=2, space="PSUM") as psB, \
             tc.tile_pool(name="psC", bufs=2, space="PSUM") as psC, \
             tc.tile_pool(name="psO", bufs=1, space="PSUM") as psO:

            c_rw1t = cpool.tile([REV_DIM, FEAT], bf16)
            nc.sync.dma_start(out=c_rw1t[:], in_=rw1t[:])
            c_rw2t8 = cpool.tile([P, 2, FEAT], fp8)
            nc.sync.dma_start(out=c_rw2t8[:], in_=rw2t8[:])
            c_w3lt = cpool.tile([P, 2, FEAT], bf16)
            nc.sync.dma_start(out=c_w3lt[:], in_=w3lt[:])
            c_lwt = cpool.tile([P, 2, FEAT], bf16)
            nc.sync.dma_start(out=c_lwt[:], in_=lwt[:])
            c_iota = cpool.tile([P, P], bf16)
            nc.sync.dma_start(out=c_iota[:], in_=iota[:])
            c_wpa = cpool.tile([P, T], f32)
            nc.sync.dma_start(out=c_wpa[:], in_=wpa[:])
            c_wra = cpool.tile([P, T], f32)
            nc.sync.dma_start(out=c_wra[:], in_=wra[:])
            c_dstr = cpool.tile([P, T], f32)
            nc.sync.dma_start(out=c_dstr[:], in_=dstr[:])

            for b in range(nb):
                s0 = b * SLOTS_B
                rts = stg.tile([P, SLOTS_B], bf16, tag="rts")
                nc.sync.dma_start(out=rts[:], in_=rth[:, s0:s0 + SLOTS_B])
                fts = stg.tile([P, TB, FEAT], bf16, tag="fts")
                nc.sync.dma_start(
                    out=fts[:],
                    in_=fth[s0:s0 + SLOTS_B, :].rearrange("(n p) d -> p n d", p=P))
                a2b = stg.tile([P, TB, FEAT], bf16, tag="a2b")

                def emit_pair(pi):
                    a1ps = psA.tile([P, 2, FEAT], f32, tag="a1ps")
                    for m in range(2):
                        nc.tensor.matmul(
                            out=a1ps[:, m, :],
                            lhsT=c_rw1t[:, m * P:(m + 1) * P],
                            rhs=rts[:, pi * 256:(pi + 1) * 256],
                            start=True, stop=True)
                    a1sb = a1pool.tile([P, 2, FEAT], fp8, tag="a1sb")
                    nc.scalar.activation(out=a1sb[:], in_=a1ps[:], func=AF.Gelu)
                    a2ps = psB.tile([P, 2, FEAT], f32, tag="a2ps")
                    for k in range(2):
                        nc.tensor.matmul(
                            out=a2ps[:, k, :],
                            lhsT=a1sb[:, :, k * P:(k + 1) * P],
                            rhs=c_rw2t8[:],
                            start=True, stop=True, perf_mode=MM.DoubleRow)
                    nc.scalar.activation(out=a2b[:, pi * 2:(pi + 1) * 2, :],
                                         in_=a2ps[:], func=AF.Gelu)

                po = None

                def emit_group(gi, po):
                    g = b * GB + gi
                    acc = psC.tile([P, 2, FEAT], f32, tag="acc")
                    # PSUM accumulation chains must be contiguous per bank
                    # region on HW: build all S tiles first, then run the
                    # four region chains back to back.
                    sas, sbs = [], []
                    for gt in range(TPG):
                        tg = b * TB + gi * TPG + gt
                        sa = msg.tile([P, P], bf16, tag="sa")
                        nc.vector.tensor_scalar(
                            out=sa[:], in0=c_iota[:],
                            scalar1=c_dstr[:, tg:tg + 1],
                            scalar2=c_wpa[:, tg:tg + 1],
                            op0=OP.is_equal, op1=OP.mult)
                        sas.append(sa)
                        sb_ = msg.tile([P, P], bf16, tag="sb")
                        nc.vector.tensor_scalar(
                            out=sb_[:], in0=c_iota[:],
                            scalar1=c_dstr[:, tg:tg + 1],
                            scalar2=c_wra[:, tg:tg + 1],
                            op0=OP.is_equal, op1=OP.mult)
                        sbs.append(sb_)
                    for f in range(2):
                        for gt in range(TPG):
                            ti = gi * TPG + gt
                            nc.tensor.matmul(
                                out=acc[:, 0, f * P:(f + 1) * P],
                                lhsT=fts[:, ti, f * P:(f + 1) * P],
                                rhs=sas[gt][:],
                                start=(gt == 0), stop=(gt == TPG - 1))
                    for j in range(2):
                        for gt in range(TPG):
                            ti = gi * TPG + gt
                            nc.tensor.matmul(
                                out=acc[:, 1, j * P:(j + 1) * P],
                                lhsT=a2b[:, ti, j * P:(j + 1) * P],
                                rhs=sbs[gt][:],
                                start=(gt == 0), stop=(gt == TPG - 1))
                    hta = drn.tile([P, FEAT], bf16, tag="hta")
                    nc.vector.tensor_copy(out=hta[:], in_=acc[:, 0, :])
                    gsb = drn.tile([P, FEAT], bf16, tag="gsb")
                    nc.vector.tensor_copy(out=gsb[:], in_=acc[:, 1, :])
                    if gi % 2 == 0:
                        po = psO.tile([P, 2, FEAT], f32, tag="po")
                    for f in range(2):
                        nc.tensor.matmul(
                            out=po[:, gi % 2, :],
                            lhsT=hta[:, f * P:(f + 1) * P],
                            rhs=c_lwt[:, f, :],
                            start=(f == 0), stop=False)
                    for j in range(2):
                        nc.tensor.matmul(
                            out=po[:, gi % 2, :],
                            lhsT=gsb[:, j * P:(j + 1) * P],
                            rhs=c_w3lt[:, j, :],
                            start=False, stop=(j == 1))
                    if gi % 2 == 1:
                        ot = otp.tile([P, 2, FEAT], bf16, tag="ot")
                        nc.scalar.activation(out=ot[:], in_=po[:], func=AF.Copy)
                        g0 = g - 1
                        nc.sync.dma_start(
                            out=outd[g0 * P:(g0 + 2) * P, :].rearrange(
                                "(n p) d -> p n d", p=P),
                            in_=ot[:])
                    return po

                # interleave 3 MLP pairs with 2 scatter groups so the PE
                # stream mixes matmul types and stays dense
                for j4 in range(GB // 2):
                    for pi in range(3 * j4, 3 * j4 + 3):
                        emit_pair(pi)
                    for gi in (2 * j4, 2 * j4 + 1):
                        po = emit_group(gi, po)
    nc.compile()
    return nc


def kernel(**inputs):
    import ml_dtypes
    from concourse.bass_utils import run_bass_kernel_spmd

    feat = np.asarray(inputs["feat"], dtype=np.float32)
    cj = np.asarray(inputs["cj"], dtype=np.float32)
    ci = np.asarray(inputs["ci"], dtype=np.float32)
    edge_src = np.asarray(inputs["edge_src"]).astype(np.int64)
    edge_dst = np.asarray(inputs["edge_dst"]).astype(np.int64)
    review_id = np.asarray(inputs["review_id"]).astype(np.int64)
    rev_emb = np.asarray(inputs["review_emb"], dtype=np.float32)
    prob_w = np.asarray(inputs["prob_w"], dtype=np.float32)
    score_w = np.asarray(inputs["score_w"], dtype=np.float32)
    rw1 = np.asarray(inputs["rw1"], dtype=np.float32)
    rw2 = np.asarray(inputs["rw2"], dtype=np.float32)
    rw3 = np.asarray(inputs["rw3"], dtype=np.float32)
    lin_w = np.asarray(inputs["lin_w"], dtype=np.float32)
    lin_b = np.asarray(inputs["lin_b"], dtype=np.float32)

    n_dst = ci.shape[0]
    bf = ml_dtypes.bfloat16

    order = np.argsort(edge_dst, kind="stable")
    s_src = edge_src[order]
    s_dst = edge_dst[order]
    s_rev = review_id[order]
    s_w = (cj[s_src, 0] * ci[s_dst, 0]).astype(np.float32)
    rfeat = rev_emb[s_rev]
    pa = 1.0 / (1.0 + np.exp(-(rfeat @ prob_w[0])))
    ra = 1.0 / (1.0 + np.exp(-(rfeat @ score_w[0])))
    s_wpa = (pa * s_w).astype(np.float32)
    s_wra = (ra * s_w).astype(np.float32)

    # nonzero dst rows in sorted order, with degree and edge offsets
    uniq_dst, deg = np.unique(s_dst, return_counts=True)
    row_end = np.cumsum(deg)            # edge offset after each row
    row_start = row_end - deg
    nrows_tot = len(uniq_dst)

    # greedy pack consecutive rows into groups: <=CAP edges, <=128 rows
    grp_row0 = []      # first row index (into uniq_dst) of each group
    grp_nrows = []
    i = 0
    while i < nrows_tot:
        base = row_start[i]
        # farthest j with row_end[j] - base <= CAP
        j = np.searchsorted(row_end, base + CAP, side="right") - 1
        j = min(max(j, i), i + P - 1)
        grp_row0.append(i)
        grp_nrows.append(j - i + 1)
        i = j + 1
    ngroups = len(grp_row0)
    ng = -(-ngroups // N_CORES)
    ng = -(-ng // GB) * GB             # groups per core, multiple of GB
    n_slots = ng * CAP
    T = ng * TPG

    # deal groups round-robin to cores: group g -> core g % 8, index g // 8
    rev16 = rev_emb.astype(bf)
    feat16 = feat.astype(bf)

    consts = dict(
        rw1t=np.ascontiguousarray(rw1.T).astype(bf),
        rw2t8=np.ascontiguousarray(
            rw2.T.reshape(2, P, FEAT).transpose(1, 0, 2)).astype(ml_dtypes.float8_e5m2),
        w3lt=np.ascontiguousarray(
            (lin_w @ rw3).T.reshape(2, P, FEAT).transpose(1, 0, 2)).astype(bf),
        lwt=np.ascontiguousarray(
            lin_w.T.reshape(2, P, FEAT).transpose(1, 0, 2)).astype(bf),
        iota=np.broadcast_to(np.arange(P), (P, P)).astype(bf).copy(),
    )

    in_maps = []
    core_meta = []
    for c in range(N_CORES):
        gl = list(range(c, ngroups, N_CORES))[:ng]
        slot_rev = np.zeros(n_slots, dtype=np.int64)
        slot_src = np.zeros(n_slots, dtype=np.int64)
        wpa_s = np.zeros(n_slots, dtype=np.float32)
        wra_s = np.zeros(n_slots, dtype=np.float32)
        dst_s = np.full(n_slots, -1.0, dtype=np.float32)
        rows_all = []
        pos_all = []
        for k, g in enumerate(gl):
            r0, nr = grp_row0[g], grp_nrows[g]
            e0, e1 = row_start[r0], row_end[r0 + nr - 1]
            n = e1 - e0
            s0 = k * CAP
            slot_rev[s0:s0 + n] = s_rev[e0:e1]
            slot_src[s0:s0 + n] = s_src[e0:e1]
            wpa_s[s0:s0 + n] = s_wpa[e0:e1]
            wra_s[s0:s0 + n] = s_wra[e0:e1]
            dst_s[s0:s0 + n] = (
                np.searchsorted(uniq_dst[r0:r0 + nr], s_dst[e0:e1])
            ).astype(np.float32)
            rows_all.append(uniq_dst[r0:r0 + nr])
            pos_all.append(k * P + np.arange(nr))
        rth = np.ascontiguousarray(rev16[slot_rev].T)           # [128, n_slots]
        fth = np.ascontiguousarray(feat16[slot_src])            # [n_slots, 256]
        im = dict(
            rth=rth, fth=fth,
            wpa=np.ascontiguousarray(wpa_s.reshape(T, P).T),
            wra=np.ascontiguousarray(wra_s.reshape(T, P).T),
            dstr=np.ascontiguousarray(dst_s.reshape(T, P).T),
            **consts)
        in_maps.append(im)
        core_meta.append((np.concatenate(rows_all) if rows_all else np.zeros(0, dtype=np.int64),
                          np.concatenate(pos_all) if pos_all else np.zeros(0, dtype=np.int64)))

    global last_inmaps, last_meta
    last_inmaps = in_maps
    last_meta = dict(ng=ng, T=T, n_slots=n_slots, core_meta=core_meta,
                     uniq_dst=uniq_dst, grp_row0=grp_row0, grp_nrows=grp_nrows)

    if ng not in _prog_cache:
        _prog_cache[ng] = _build_program(ng)
    nc = _prog_cache[ng]

    trace = bool(os.environ.get("BASS_KERNEL_TRACE"))
    res = run_bass_kernel_spmd(nc, in_maps, core_ids=list(range(N_CORES)),
                               trace=trace)
    global last_results
    last_results = res

    out = np.broadcast_to(lin_b, (n_dst, FEAT)).astype(np.float32).copy()
    for c in range(N_CORES):
        rows, pos = core_meta[c]
        if len(rows):
            out[rows] = res.results[c]["outd"][pos].astype(np.float32) + lin_b
    return out


last_results = None
last_inmaps = None
last_meta = None
